# revision 1
# baseline (speedup 1.0000x reference)
"""Trainium2 Bass kernel for nn_AddNet (2-layer gated RNN, T=1024 B=64 INP=512 HS=1024 OUT=512).

Algorithm restructure: the gates a1=sigmoid(x@Wi1.T), a2=sigmoid(a1@Wi2.T) depend
only on the input sequence, never on recurrent state. So all four big matmuls
batch over all (t, b) rows; the only sequential pieces are first-order linear
scans h[t] = c[t]*h[t-1] + u[t], done with the hardware tensor_tensor_scan
instruction (fp32 internal state) on the vector engine.

Sharding: data-parallel over batch B across the 8 NeuronCores (8 batch rows per
core, weights replicated, no collectives).

Device layout is "transposed world": every activation lives as [feature dim on
partitions, (b, t) on free] so matmul outputs chain directly into the next
matmul's moving operand and into per-(feature,b) time scans.
"""

import os
import sys

import numpy as np
import ml_dtypes

for _p in ("/root/.axon_site/_ro/trn_rl_repo", "/opt/trn_rl_repo"):
    if os.path.isdir(_p) and _p not in sys.path:
        sys.path.append(_p)

import concourse.bass as bass  # noqa: E402
import concourse.tile as tile  # noqa: E402
from concourse import bacc, mybir  # noqa: E402
from concourse.bass_utils import run_bass_kernel_spmd  # noqa: E402

# If tracing is requested (BASS_TRACE) in an image whose antenv stub lacks
# axon_hooks, run_bass_kernel_spmd would crash on import. Register a minimal
# fallback registry so the trace path degrades gracefully instead.
try:  # noqa: SIM105
    import antenv.axon_hooks  # noqa: F401
except ImportError:
    import types as _types

    _ah = _types.ModuleType("antenv.axon_hooks")
    _ah._hook = None
    _ah.set_axon_ntff_profile_hook = lambda h: setattr(_ah, "_hook", h)
    _ah.get_axon_ntff_profile_hook = lambda: _ah._hook
    sys.modules["antenv.axon_hooks"] = _ah
    try:
        import antenv as _antenv

        _antenv.axon_hooks = _ah
        from trn_agent_boot.trn_boot import _ntff_profile_via_ctypes

        if os.path.exists("/opt/axon/libaxon_pjrt.so"):
            _ah.set_axon_ntff_profile_hook(
                _ntff_profile_via_ctypes("/opt/axon/libaxon_pjrt.so"))
    except Exception:  # noqa: BLE001
        pass

F32 = mybir.dt.float32
BF16 = mybir.dt.bfloat16
AF = mybir.ActivationFunctionType
OP = mybir.AluOpType
BF = ml_dtypes.bfloat16

T, B, INP, HS, OUT = 1024, 64, 512, 1024, 512
NCORES = 8
BL = B // NCORES

# consts tensor column layout: per-partition scalars for each HS (8) / OUT (4) tile
_C_THR1, _C_DR1, _C_THR2, _C_DR2, _C_BI1, _C_B1, _C_B2, _C_B3 = (
    0, 8, 16, 24, 32, 40, 48, 56,
)
_C_W = 60


def build(T_=T, BL_=BL, INP_=INP, HS_=HS, OUT_=OUT, b1_nz=False, b2_nz=False):
    """Build + compile the per-core Bass program (SPMD: same graph on all cores)."""
    KX = INP_ // 128   # k-tiles of the input dim
    KH = HS_ // 128    # k-tiles / m-tiles of the hidden dim
    MO = OUT_ // 128
    HC = min(512, T_)  # psum column-chunk width
    NCH = T_ // HC

    nc = bacc.Bacc("TRN2", target_bir_lowering=False, debug=False,
                   num_devices=NCORES)

    xt_d = nc.dram_tensor("xt", [INP_, BL_ * T_], BF16, kind="ExternalInput")
    w12_d = nc.dram_tensor("w12t", [INP_, 2 * HS_], BF16, kind="ExternalInput")
    wi2_d = nc.dram_tensor("wi2t", [HS_, HS_], BF16, kind="ExternalInput")
    w2_d = nc.dram_tensor("w2t", [HS_, HS_], BF16, kind="ExternalInput")
    w3_d = nc.dram_tensor("w3t", [HS_, OUT_], BF16, kind="ExternalInput")
    cst_d = nc.dram_tensor("consts", [128, _C_W], F32, kind="ExternalInput")
    out_d = nc.dram_tensor("out", [OUT_, BL_], F32, kind="ExternalOutput")

    with tile.TileContext(nc) as tc, \
         tc.tile_pool(name="persist", bufs=1) as pp, \
         tc.tile_pool(name="xtp", bufs=2) as xtp, \
         tc.tile_pool(name="a1p", bufs=2) as a1p, \
         tc.tile_pool(name="anp", bufs=2) as anp, \
         tc.tile_pool(name="c2p", bufs=2) as c2p, \
         tc.tile_pool(name="c1p", bufs=3) as c1p, \
         tc.tile_pool(name="h1p", bufs=3) as h1p, \
         tc.tile_pool(name="s2p", bufs=3) as s2p, \
         tc.tile_pool(name="u1p", bufs=2) as u1p, \
         tc.tile_pool(name="psA", bufs=3, space="PSUM") as psA, \
         tc.tile_pool(name="psB", bufs=2, space="PSUM") as psB, \
         tc.tile_pool(name="psC", bufs=3, space="PSUM") as psC:

        # DMA order matters for the kernel head: the first matmuls need only
        # w12 + xt(b=0). xt and consts ride the gpsimd queue so they never
        # wait behind weights on the sync queue; w3 (needed only by the
        # readout) goes last.
        w12 = pp.tile([128, KX * 2 * HS_], BF16, tag="w12")
        for k in range(KX):
            nc.sync.dma_start(w12[:, k * 2 * HS_:(k + 1) * 2 * HS_],
                              w12_d.ap()[k * 128:(k + 1) * 128, :])
        cst = pp.tile([128, _C_W], F32, tag="cst")
        nc.gpsimd.dma_start(cst[:, :], cst_d.ap()[:, :])
        wi2 = pp.tile([128, KH * HS_], BF16, tag="wi2")
        w2 = pp.tile([128, KH * HS_], BF16, tag="w2")
        for k in range(KH):
            nc.sync.dma_start(wi2[:, k * HS_:(k + 1) * HS_],
                              wi2_d.ap()[k * 128:(k + 1) * 128, :])
        for k in range(KH):
            nc.sync.dma_start(w2[:, k * HS_:(k + 1) * HS_],
                              w2_d.ap()[k * 128:(k + 1) * 128, :])
        w3 = pp.tile([128, KH * OUT_], BF16, tag="w3")
        for k in range(KH):
            nc.sync.dma_start(w3[:, k * OUT_:(k + 1) * OUT_],
                              w3_d.ap()[k * 128:(k + 1) * 128, :])

        h2f = pp.tile([128, KH * BL_], F32, tag="h2f")

        # PE warm-up: ~3.4us of dummy matmuls inside the DMA-wait head flips
        # the HAM clock-gate to 8/8 before the first real matmul, so the real
        # stream never pays the 1.2 GHz cold ramp. Reads a memset scratch tile
        # (no DMA dependency); the psum scratch is never read.
        if T_ >= 1024:
            wu = pp.tile([128, 128], BF16, tag="warm")
            nc.gpsimd.memset(wu[:, :], 0.0)
            pw = psA.tile([128, 512], F32, tag="psA")
            for g in range(8):
                for k in range(4):
                    nc.tensor.matmul(pw[:, 0:128], wu[:, :], wu[:, :],
                                     start=(k == 0), stop=(k == 3))

        def csc(base, j):  # per-partition scalar AP from the consts tile
            return cst[:, base + j:base + j + 1]

        for b in range(BL_):
            xt = xtp.tile([128, KX * T_], BF16, tag="xt")
            for h in range(NCH):
                for k in range(KX):
                    nc.gpsimd.dma_start(
                        xt[:, k * T_ + h * HC:k * T_ + (h + 1) * HC],
                        xt_d.ap()[k * 128:(k + 1) * 128,
                                  b * T_ + h * HC:b * T_ + (h + 1) * HC])

            a1 = a1p.tile([128, KH * T_], BF16, tag="a1")
            an1 = anp.tile([128, KH * T_], BF16, tag="an1")
            c2 = c2p.tile([128, KH * T_], BF16, tag="c2")

            # ---- phase A: z1 -> a1, c1 ; u1 -> scan1 -> h1 -> tanh -> an1
            for j in range(KH):
                c1 = c1p.tile([128, T_], BF16, tag="c1")
                for h in range(NCH):
                    cs = slice(h * HC, (h + 1) * HC)
                    pz = psA.tile([128, HC], F32, tag="psA")
                    for k in range(KX):
                        nc.tensor.matmul(
                            pz[:, :],
                            w12[:, k * 2 * HS_ + j * 128:k * 2 * HS_ + (j + 1) * 128],
                            xt[:, k * T_ + h * HC:k * T_ + (h + 1) * HC],
                            start=(k == 0), stop=(k == KX - 1))
                    nc.scalar.activation(a1[:, j * T_ + h * HC:j * T_ + (h + 1) * HC],
                                         pz[:, :], AF.Sigmoid, bias=csc(_C_BI1, j))
                    nc.vector.tensor_scalar(c1[:, cs], pz[:, :],
                                            csc(_C_THR1, j), csc(_C_DR1, j),
                                            op0=OP.is_gt, op1=OP.mult)
                h1 = h1p.tile([128, T_], BF16, tag="h1")
                for h in range(NCH):
                    cs = slice(h * HC, (h + 1) * HC)
                    pu = psA.tile([128, HC], F32, tag="psA")
                    for k in range(KX):
                        nc.tensor.matmul(
                            pu[:, :],
                            w12[:, k * 2 * HS_ + HS_ + j * 128:
                                   k * 2 * HS_ + HS_ + (j + 1) * 128],
                            xt[:, k * T_ + h * HC:k * T_ + (h + 1) * HC],
                            start=(k == 0), stop=(k == KX - 1))
                    scan_src = pu[:, :]
                    if b1_nz:
                        u1s = u1p.tile([128, HC], F32, tag="u1s")
                        nc.vector.tensor_scalar_add(u1s[:, :], pu[:, :], csc(_C_B1, j))
                        scan_src = u1s[:, :]
                    nc.vector.tensor_tensor_scan(
                        h1[:, cs], c1[:, cs], scan_src,
                        initial=(0.0 if h == 0 else h1[:, h * HC - 1:h * HC]),
                        op0=OP.mult, op1=OP.add)
                    nc.scalar.activation(an1[:, j * T_ + h * HC:j * T_ + (h + 1) * HC],
                                         h1[:, cs], AF.Tanh)

            # ---- phase B: z2 = a1 @ Wi2.T -> c2
            for j in range(KH):
                for h in range(NCH):
                    cs = slice(h * HC, (h + 1) * HC)
                    pz2 = psB.tile([128, HC], F32, tag="psB")
                    for k in range(KH):
                        nc.tensor.matmul(
                            pz2[:, :],
                            wi2[:, k * HS_ + j * 128:k * HS_ + (j + 1) * 128],
                            a1[:, k * T_ + h * HC:k * T_ + (h + 1) * HC],
                            start=(k == 0), stop=(k == KH - 1))
                    nc.vector.tensor_scalar(c2[:, j * T_ + h * HC:j * T_ + (h + 1) * HC],
                                            pz2[:, :],
                                            csc(_C_THR2, j), csc(_C_DR2, j),
                                            op0=OP.is_gt, op1=OP.mult)

            # ---- phase C: v2 = an1 @ W2.T -> scan2 -> h2 final column
            for j in range(KH):
                s2 = s2p.tile([128, T_], F32, tag="s2")
                for h in range(NCH):
                    cs = slice(h * HC, (h + 1) * HC)
                    pv = psC.tile([128, HC], F32, tag="psC")
                    for k in range(KH):
                        nc.tensor.matmul(
                            pv[:, :],
                            w2[:, k * HS_ + j * 128:k * HS_ + (j + 1) * 128],
                            an1[:, k * T_ + h * HC:k * T_ + (h + 1) * HC],
                            start=(k == 0), stop=(k == KH - 1))
                    scan_src = pv[:, :]
                    if b2_nz:
                        u2s = u1p.tile([128, HC], F32, tag="u2s")
                        nc.vector.tensor_scalar_add(u2s[:, :], pv[:, :], csc(_C_B2, j))
                        scan_src = u2s[:, :]
                    nc.vector.tensor_tensor_scan(
                        s2[:, cs], c2[:, j * T_ + h * HC:j * T_ + (h + 1) * HC],
                        scan_src,
                        initial=(0.0 if h == 0 else s2[:, h * HC - 1:h * HC]),
                        op0=OP.mult, op1=OP.add)
                nc.gpsimd.tensor_copy(h2f[:, j * BL_ + b:j * BL_ + b + 1],
                                      s2[:, T_ - 1:T_])

        # ---- readout: out.T = W3 @ tanh(h2f) (+ b3)
        an2 = pp.tile([128, KH * BL_], BF16, tag="an2")
        for j in range(KH):
            nc.scalar.activation(an2[:, j * BL_:(j + 1) * BL_],
                                 h2f[:, j * BL_:(j + 1) * BL_], AF.Tanh)
        outsb = pp.tile([128, MO * BL_], F32, tag="outsb")
        for mo in range(MO):
            po = psA.tile([128, BL_], F32, tag="psA")
            for k in range(KH):
                nc.tensor.matmul(po[:, :],
                                 w3[:, k * OUT_ + mo * 128:k * OUT_ + (mo + 1) * 128],
                                 an2[:, k * BL_:(k + 1) * BL_],
                                 start=(k == 0), stop=(k == KH - 1))
            nc.vector.tensor_scalar_add(outsb[:, mo * BL_:(mo + 1) * BL_],
                                        po[:, :], csc(_C_B3, mo))
        for mo in range(MO):
            nc.sync.dma_start(out_d.ap()[mo * 128:(mo + 1) * 128, :],
                              outsb[:, mo * BL_:(mo + 1) * BL_])

    nc.compile()
    return nc


def _host_prep(inputs, T_=T, B_=B, INP_=INP, HS_=HS, OUT_=OUT, ncores=NCORES):
    """Host-side sharding / transposition / packing. Not on the device clock."""
    f32 = np.float32
    data = np.asarray(inputs["data"], f32)
    W1, b1 = np.asarray(inputs["W1"], f32), np.asarray(inputs["b1"], f32)
    Wi1, bi1 = np.asarray(inputs["Wi1"], f32), np.asarray(inputs["bi1"], f32)
    t1, dr1 = np.asarray(inputs["t1"], np.float64), np.asarray(inputs["dr1"], f32)
    W2, b2 = np.asarray(inputs["W2"], f32), np.asarray(inputs["b2"], f32)
    Wi2, bi2 = np.asarray(inputs["Wi2"], f32), np.asarray(inputs["bi2"], f32)
    t2, dr2 = np.asarray(inputs["t2"], np.float64), np.asarray(inputs["dr2"], f32)
    W3, b3 = np.asarray(inputs["W3"], f32), np.asarray(inputs["b3"], f32)

    KH = HS_ // 128
    MO = OUT_ // 128
    bl = B_ // ncores

    w12t = np.concatenate([Wi1.T, W1.T], axis=1).astype(BF)   # [INP, 2*HS]
    wi2t = np.ascontiguousarray(Wi2.T).astype(BF)
    w2t = np.ascontiguousarray(W2.T).astype(BF)
    w3t = np.ascontiguousarray(W3.T).astype(BF)

    # gate thresholds in pre-activation space: sigmoid(z+bi) > t  <=>  z > logit(t)-bi
    thr1 = (np.log(t1 / (1.0 - t1)) - bi1).astype(f32)
    thr2 = (np.log(t2 / (1.0 - t2)) - bi2).astype(f32)

    cst = np.zeros((128, _C_W), f32)
    col = lambda v, n: np.asarray(v, f32).reshape(n, 128).T
    cst[:, _C_THR1:_C_THR1 + KH] = col(thr1, KH)
    cst[:, _C_DR1:_C_DR1 + KH] = col(dr1, KH)
    cst[:, _C_THR2:_C_THR2 + KH] = col(thr2, KH)
    cst[:, _C_DR2:_C_DR2 + KH] = col(dr2, KH)
    cst[:, _C_BI1:_C_BI1 + KH] = col(bi1, KH)
    cst[:, _C_B1:_C_B1 + KH] = col(b1, KH)
    cst[:, _C_B2:_C_B2 + KH] = col(b2, KH)
    cst[:, _C_B3:_C_B3 + MO] = col(b3, MO)

    in_maps = []
    for c in range(ncores):
        sh = data[:, c * bl:(c + 1) * bl, :]          # [T, bl, INP]
        xt = sh.transpose(2, 1, 0).reshape(INP_, bl * T_).astype(BF)
        in_maps.append({"xt": xt, "w12t": w12t, "wi2t": wi2t, "w2t": w2t,
                        "w3t": w3t, "consts": cst})
    flags = dict(b1_nz=bool(np.any(b1)), b2_nz=bool(np.any(b2)))
    return in_maps, flags


_NC_CACHE = {}
LAST_RESULT = {}


def kernel(**inputs):
    in_maps, flags = _host_prep(inputs)
    key = tuple(sorted(flags.items()))
    if key not in _NC_CACHE:
        _NC_CACHE[key] = build(**flags)
    nc = _NC_CACHE[key]
    kw = {}
    if os.environ.get("KERNEL_TRACE_DIR"):
        kw["tmpdir"] = os.environ["KERNEL_TRACE_DIR"]
        kw["trace"] = True
    res = run_bass_kernel_spmd(nc, in_maps, core_ids=list(range(NCORES)), **kw)
    LAST_RESULT["res"] = res
    out = np.empty((B, OUT), np.float32)
    bl = B // NCORES
    for c in range(NCORES):
        out[c * bl:(c + 1) * bl, :] = np.asarray(res.results[c]["out"],
                                                 np.float32).T
    return out

